# revision 9
# baseline (speedup 1.0000x reference)
"""Multi-head attention (B=4, S=2048, D=1024, H=16, E=64) on 8 TRN2 NeuronCores.

Sharding: core c handles batch b=c//2 and query-half qh=c%2 (1024 query tokens).
K/V are computed per-core for the full 2048-token sequence of its batch (2x
duplicated K/V projection work, but zero collectives / zero cross-core deps).

The host reorders each core's xT so the core's OWN query tokens occupy columns
0..1023 (attention is invariant to key/value token order as long as K and V
share it), so a single SPMD program serves all cores without a separate xTq
input.

Per-core program (SPMD, identical on all cores):
  V projection (4 head-quads of 256 cols): V = x @ wv + bv, stored
    [tok, head, 65] with a ones-column per head (softmax sums fall out of the
    att@V matmul), spilled to DRAM scratch. Quads 0-1 run up front; quads 2-3
    (heads 8-15, first needed in pass 4) are emitted after passes 0/1 so the
    Tile scheduler uses them as PE filler inside the exp-bound inner loop.
  passes p=0..7 (heads 2p, 2p+1):
    KT[128he, 2048tok] = (wk_p.T @ xT) + bk  (transposed layout, bias on DVE)
    QT[128he, 1024tq]  = (wq_p.T @ xT[:, :1024]) + bq
    per (head, tq-tile of 512):
      scoresT[tk,tq] = KT_h.T-slices @ QT_h  (K=64 matmuls, one per tk-tile)
      exp on ScalarE straight from PSUM with scale=1/8 (softmax max-subtraction
      skipped: |score/8| <= ~12 so exp is fp32-safe)
      attT[65,tq] += [V_h | 1].T @ expT  (row 64 accumulates the softmax sum)
      normalize: reciprocal_approx on VectorE, partition_broadcast on GpSimd,
      multiply on VectorE — the PE is never involved
  output projection (4 col-quads of 256): out = att @ wo.T + bo, bias added on
    DVE from a partition-broadcast bo tile.

Double-buffered wkq/kt/qt pools let the scheduler run the NEXT pass's K/Q
projections as PE filler during the current pass's attention inner loop, which
is otherwise rate-limited by ScalarE exp (~1.1us per 2-ktile group) and stalls
the PE, dropping its p-state.

All matmuls run in float32r (tf32-like, full PE rate at N>=256).
"""

import numpy as np

import concourse.bass as bass
import concourse.mybir as mybir
import concourse.tile as tile
from concourse import bacc
from concourse.bass_utils import run_bass_kernel_spmd

FP32 = mybir.dt.float32
FP32R = mybir.dt.float32r
AF = mybir.ActivationFunctionType

B, S, D, H, E = 4, 2048, 1024, 16, 64
NCORES = 8
TQ = S // 2  # query tokens per core
SCALE = 1.0 / float(np.sqrt(E))

_CACHE = {}


def build_nc():
    nc = bacc.Bacc("TRN2", target_bir_lowering=False)

    xT = nc.dram_tensor("xT", [D, S], FP32R, kind="ExternalInput")
    wq_t = nc.dram_tensor("wq_t", [D, H * E], FP32R, kind="ExternalInput")
    wk_t = nc.dram_tensor("wk_t", [D, H * E], FP32R, kind="ExternalInput")
    wv_t = nc.dram_tensor("wv_t", [D, H * E], FP32R, kind="ExternalInput")
    wo_t = nc.dram_tensor("wo_t", [D, D], FP32R, kind="ExternalInput")
    bqp = nc.dram_tensor("bqp", [128, 8], FP32, kind="ExternalInput")
    bkp = nc.dram_tensor("bkp", [128, 8], FP32, kind="ExternalInput")
    bv_row = nc.dram_tensor("bv_row", [1, H * E], FP32R, kind="ExternalInput")
    bo_row = nc.dram_tensor("bo_row", [1, D], FP32R, kind="ExternalInput")
    out = nc.dram_tensor("out", [TQ, D], FP32, kind="ExternalOutput")
    # V spill: [tok-tile, tok-in-tile, head, E+1]
    v_spill = nc.dram_tensor("v_spill", [16, 128, H, E + 1], FP32R)

    xT_r = xT.rearrange("(t p) s -> p t s", p=128)  # [128, 8, 2048]
    wq_r = wq_t.rearrange("(t p) m -> p t m", p=128)  # [128, 8, 1024]
    wk_r = wk_t.rearrange("(t p) m -> p t m", p=128)
    wv_r = wv_t.rearrange("(t p) m -> p t m", p=128)
    wo_r = wo_t.rearrange("(t p) m -> p t m", p=128)

    with tile.TileContext(nc) as tc:
        with (
            tc.tile_pool(name="xt", bufs=1) as xt_pool,
            tc.tile_pool(name="wkq", bufs=2) as wkq_pool,
            tc.tile_pool(name="ktp", bufs=2) as kt_pool,
            tc.tile_pool(name="w256", bufs=2) as w256_pool,
            tc.tile_pool(name="qt", bufs=2) as qt_pool,
            tc.tile_pool(name="vbuf", bufs=2) as vbuf_pool,
            tc.tile_pool(name="vst", bufs=2) as vst_pool,
            tc.tile_pool(name="expp", bufs=2) as exp_pool,
            tc.tile_pool(name="attT", bufs=8) as attT_pool,
            tc.tile_pool(name="small", bufs=2) as small_pool,
            tc.tile_pool(name="ones", bufs=1) as ones_pool,
            tc.tile_pool(name="ps_s", bufs=2, space="PSUM") as ps_scores,
            tc.tile_pool(name="ps_a", bufs=2, space="PSUM") as ps_att,
            tc.tile_pool(name="ps_g", bufs=2, space="PSUM") as ps_gen,
        ):
            # ---- persistent tiles ----
            xt_sb = xt_pool.tile([128, 8, S], FP32R, tag="xt")  # 64KB/part
            nc.sync.dma_start(out=xt_sb, in_=xT_r)

            ones_col_f = ones_pool.tile([128, 8], FP32, tag="onescf")
            nc.vector.memset(ones_col_f, 1.0)
            ones_col = ones_pool.tile([128, 8], FP32R, tag="onescol")
            nc.vector.tensor_copy(out=ones_col, in_=ones_col_f)
            bq_sb = ones_pool.tile([128, 8], FP32, tag="bq")
            bk_sb = ones_pool.tile([128, 8], FP32, tag="bk")
            nc.sync.dma_start(out=bq_sb, in_=bqp[:, :])
            nc.sync.dma_start(out=bk_sb, in_=bkp[:, :])

            # bias rows staged through the w256 pool (dead after broadcast)
            bv_sb = w256_pool.tile([1, H * E], FP32R, tag="w256", name="bvrow")
            bo_sb = w256_pool.tile([1, D], FP32R, tag="w256", name="borow")
            nc.sync.dma_start(out=bv_sb, in_=bv_row[:, :])
            nc.sync.dma_start(out=bo_sb, in_=bo_row[:, :])
            bv_bc = ones_pool.tile([128, H * E], FP32R, tag="bvbc")
            bo_bc = ones_pool.tile([128, D], FP32R, tag="bobc")
            nc.gpsimd.partition_broadcast(bv_bc, bv_sb)
            nc.gpsimd.partition_broadcast(bo_bc, bo_sb)

            attT_tiles = [
                attT_pool.tile([128, TQ], FP32R, tag="attT", name=f"attT{i}")
                for i in range(8)
            ]

            # ---- V projection for one head-quad (4 heads, 256 cols) ----
            def v_quad(vh):
                wv_sb = w256_pool.tile([128, 8, 256], FP32R, tag="w256")
                nc.sync.dma_start(out=wv_sb, in_=wv_r[:, :, vh * 256 : (vh + 1) * 256])
                for tokt in range(16):
                    ps = ps_gen.tile([128, 256], FP32, tag="gen")
                    for k in range(8):
                        nc.tensor.matmul(
                            out=ps,
                            lhsT=xt_sb[:, k, tokt * 128 : (tokt + 1) * 128],
                            rhs=wv_sb[:, k, :],
                            start=(k == 0),
                            stop=(k == 7),
                        )
                    vstage = vst_pool.tile([128, 4, E + 1], FP32R, tag="vst")
                    nc.vector.tensor_add(
                        out=vstage[:, :, :E],
                        in0=ps.rearrange("p (h e) -> p h e", e=E),
                        in1=bv_bc[:, vh * 256 : (vh + 1) * 256].rearrange(
                            "p (h e) -> p h e", e=E
                        ),
                    )
                    nc.vector.tensor_copy(
                        out=vstage[:, :, E : E + 1], in_=ones_col[:, :4].unsqueeze(2)
                    )
                    nc.sync.dma_start(
                        out=v_spill[tokt, :, vh * 4 : (vh + 1) * 4, :], in_=vstage
                    )

            # heads 0-7 are needed from pass 0: project them up front
            v_quad(0)
            v_quad(1)

            # ---- passes: 2 heads each ----
            for p in range(8):
                wk_sb = wkq_pool.tile([128, 8, 128], FP32R, tag="wk")
                wq_sb = wkq_pool.tile([128, 8, 128], FP32R, tag="wq")
                nc.sync.dma_start(out=wk_sb, in_=wk_r[:, :, p * 128 : (p + 1) * 128])
                nc.sync.dma_start(out=wq_sb, in_=wq_r[:, :, p * 128 : (p + 1) * 128])

                kt_sb = kt_pool.tile([128, S], FP32R, tag="kt")
                qt_sb = qt_pool.tile([128, TQ], FP32R, tag="qt")

                for ts in range(4):
                    ps = ps_gen.tile([128, 512], FP32, tag="gen")
                    for k in range(8):
                        nc.tensor.matmul(
                            out=ps,
                            lhsT=wk_sb[:, k, :],
                            rhs=xt_sb[:, k, ts * 512 : (ts + 1) * 512],
                            start=(k == 0),
                            stop=(k == 7),
                        )
                    nc.vector.tensor_scalar_add(
                        out=kt_sb[:, ts * 512 : (ts + 1) * 512],
                        in0=ps,
                        scalar1=bk_sb[:, p : p + 1],
                    )
                for qs in range(2):
                    ps = ps_gen.tile([128, 512], FP32, tag="gen")
                    for k in range(8):
                        nc.tensor.matmul(
                            out=ps,
                            lhsT=wq_sb[:, k, :],
                            rhs=xt_sb[:, k, qs * 512 : (qs + 1) * 512],
                            start=(k == 0),
                            stop=(k == 7),
                        )
                    nc.vector.tensor_scalar_add(
                        out=qt_sb[:, qs * 512 : (qs + 1) * 512],
                        in0=ps,
                        scalar1=bq_sb[:, p : p + 1],
                    )

                for hh in range(2):
                    base = hh * 64
                    h = 2 * p + hh
                    vh_sb = vbuf_pool.tile([128, 16, E + 1], FP32R, tag="vbuf")
                    nc.sync.dma_start(
                        out=vh_sb, in_=v_spill[:, :, h, :].transpose([1, 0, 2])
                    )
                    for tqt in range(2):
                        att_ps = ps_att.tile([E + 1, 512], FP32, tag="att")
                        for g in range(8):
                            ps_s = ps_scores.tile([128, 2, 512], FP32, tag="sc")
                            for j in range(2):
                                t = g * 2 + j
                                nc.tensor.matmul(
                                    out=ps_s[:, j, :],
                                    lhsT=kt_sb[
                                        base : base + 64, t * 128 : (t + 1) * 128
                                    ],
                                    rhs=qt_sb[
                                        base : base + 64, tqt * 512 : (tqt + 1) * 512
                                    ],
                                    start=True,
                                    stop=True,
                                )
                            exp_t = exp_pool.tile([128, 2, 512], FP32R, tag="exp")
                            nc.scalar.activation(
                                out=exp_t, in_=ps_s, func=AF.Exp, scale=SCALE
                            )
                            for j in range(2):
                                t = g * 2 + j
                                nc.tensor.matmul(
                                    out=att_ps,
                                    lhsT=vh_sb[:, t, :],
                                    rhs=exp_t[:, j, :],
                                    start=(t == 0),
                                    stop=(t == 15),
                                )
                        # the custom-DVE reciprocal ignores the AP's partition
                        # offset, so first move the sums row (PSUM partition
                        # 64) to a partition-0 SBUF tile with a plain copy
                        sums_sb = small_pool.tile([1, 512], FP32, tag="sums", bufs=2)
                        nc.vector.tensor_copy(out=sums_sb, in_=att_ps[E : E + 1, :])
                        recip_r = small_pool.tile([1, 512], FP32, tag="recr", bufs=2)
                        recip_s = small_pool.tile([1, 512], FP32, tag="recs", bufs=2)
                        nc.vector.reciprocal_approx_accurate(
                            out=recip_r, in_=sums_sb, scratch=recip_s
                        )
                        rb_sb = small_pool.tile([64, 512], FP32, tag="rbb", bufs=2)
                        nc.gpsimd.partition_broadcast(rb_sb, recip_r)
                        nc.vector.tensor_mul(
                            out=attT_tiles[p][
                                base : base + 64, tqt * 512 : (tqt + 1) * 512
                            ],
                            in0=att_ps[:E, :],
                            in1=rb_sb,
                        )

                # defer V projection of heads 8-15 into the exp-bound passes:
                # quad 2 (heads 8-11) lands after pass 0, quad 3 after pass 1
                if p == 0:
                    v_quad(2)
                elif p == 1:
                    v_quad(3)

            # ---- output projection (4 col-quads of 256) ----
            for ohalf in range(4):
                wo_sb = w256_pool.tile([128, 8, 256], FP32R, tag="w256")
                nc.sync.dma_start(
                    out=wo_sb, in_=wo_r[:, :, ohalf * 256 : (ohalf + 1) * 256]
                )
                for tokt in range(8):
                    ps = ps_gen.tile([128, 256], FP32, tag="gen")
                    for t in range(8):
                        nc.tensor.matmul(
                            out=ps,
                            lhsT=attT_tiles[t][:, tokt * 128 : (tokt + 1) * 128],
                            rhs=wo_sb[:, t, :],
                            start=(t == 0),
                            stop=(t == 7),
                        )
                    ostg = small_pool.tile([128, 256], FP32, tag="stg", bufs=2)
                    nc.vector.tensor_add(
                        out=ostg, in0=ps, in1=bo_bc[:, ohalf * 256 : (ohalf + 1) * 256]
                    )
                    nc.sync.dma_start(
                        out=out[
                            tokt * 128 : (tokt + 1) * 128,
                            ohalf * 256 : (ohalf + 1) * 256,
                        ],
                        in_=ostg,
                    )

    nc.compile()
    return nc


def kernel(x, wq, bq, wk, bk, wv, bv, wo, bo, trace=False):
    x = np.asarray(x, dtype=np.float32)
    wq = np.asarray(wq, dtype=np.float32)
    bq = np.asarray(bq, dtype=np.float32)
    wk = np.asarray(wk, dtype=np.float32)
    bk = np.asarray(bk, dtype=np.float32)
    wv = np.asarray(wv, dtype=np.float32)
    bv = np.asarray(bv, dtype=np.float32)
    wo = np.asarray(wo, dtype=np.float32)
    bo = np.asarray(bo, dtype=np.float32)

    if "nc" not in _CACHE:
        _CACHE["nc"] = build_nc()
    nc = _CACHE["nc"]

    wq_t = np.ascontiguousarray(wq.transpose(1, 0, 2).reshape(D, H * E))
    wk_t = np.ascontiguousarray(wk.transpose(1, 0, 2).reshape(D, H * E))
    wv_t = np.ascontiguousarray(wv.transpose(1, 0, 2).reshape(D, H * E))
    wo_t = np.ascontiguousarray(wo.T)
    bqp = np.ascontiguousarray(bq.reshape(H * E).reshape(8, 128).T)
    bkp = np.ascontiguousarray(bk.reshape(H * E).reshape(8, 128).T)
    bv_row = np.ascontiguousarray(bv.reshape(1, H * E))
    bo_row = np.ascontiguousarray(bo.reshape(1, D))

    shared = {
        "wq_t": wq_t,
        "wk_t": wk_t,
        "wv_t": wv_t,
        "wo_t": wo_t,
        "bqp": bqp,
        "bkp": bkp,
        "bv_row": bv_row,
        "bo_row": bo_row,
    }
    in_maps = []
    for c in range(NCORES):
        b, qh = c // 2, c % 2
        xT_b = x[b].T  # [D, S]
        if qh == 0:
            xT_c = np.ascontiguousarray(xT_b)
        else:
            # my query tokens first; K/V token order is irrelevant as long as
            # K and V agree (softmax + weighted sum are permutation-invariant)
            xT_c = np.ascontiguousarray(
                np.concatenate([xT_b[:, TQ:], xT_b[:, :TQ]], axis=1)
            )
        m = dict(shared)
        m["xT"] = xT_c
        in_maps.append(m)

    res = run_bass_kernel_spmd(nc, in_maps, list(range(NCORES)), trace=trace)

    out = np.empty((B, S, D), dtype=np.float32)
    for c in range(NCORES):
        b, qh = c // 2, c % 2
        out[b, qh * TQ : (qh + 1) * TQ, :] = res.results[c]["out"]
    if trace:
        return out, res
    return out


# revision 15
# speedup vs baseline: 1.2061x; 1.2061x over previous
"""Multi-head attention (B=4, S=2048, D=1024, H=16, E=64) on 8 TRN2 NeuronCores.

Sharding: core c handles batch b=c//2 and query-half qh=c%2 (1024 query tokens).
K/V are computed per-core for the full 2048-token sequence of its batch (2x
duplicated K/V projection work, but zero collectives / zero cross-core deps).

The host reorders each core's xT so the core's OWN query tokens occupy columns
0..1023 (attention is invariant to key/value token order as long as K and V
share it), so a single SPMD program serves all cores without a separate xTq
input.

Per-core program (SPMD, identical on all cores):
  V projection (4 head-quads of 256 cols): V = x @ wv + bv, stored
    [tok, head, 65] with a ones-column per head (softmax sums fall out of the
    att@V matmul), spilled to DRAM scratch. Quads 0-1 run up front; quads 2-3
    (heads 8-15, first needed in pass 4) are emitted after passes 0/1 so the
    Tile scheduler uses them as PE filler inside the exp-bound inner loop.
  passes p=0..7 (heads 2p, 2p+1):
    KT[128he, 2048tok] = (wk_p.T @ xT) + bk  (transposed layout, bias on DVE)
    QT[128he, 1024tq]  = (wq_p.T @ xT[:, :1024]) + bq
    per (head, tq-tile of 512):
      scoresT[tk,tq] = KT_h.T-slices @ QT_h  (K=64 matmuls, one per tk-tile)
      exp on ScalarE straight from PSUM with scale=1/8 (softmax max-subtraction
      skipped: |score/8| <= ~12 so exp is fp32-safe)
      attT[65,tq] += [V_h | 1].T @ expT  (row 64 accumulates the softmax sum)
      normalize: reciprocal_approx on VectorE, partition_broadcast on GpSimd,
      multiply on VectorE — the PE is never involved
  output projection (4 col-quads of 256): out = att @ wo.T + bo, bias added on
    DVE from a partition-broadcast bo tile.

Double-buffered wkq/kt/qt pools let the scheduler run the NEXT pass's K/Q
projections as PE filler during the current pass's attention inner loop, which
is otherwise rate-limited by ScalarE exp (~1.1us per 2-ktile group) and stalls
the PE, dropping its p-state.

All matmuls run in float32r (tf32-like, full PE rate at N>=256).
"""

import numpy as np

import concourse.bass as bass
import concourse.mybir as mybir
import concourse.tile as tile
from concourse import bacc
from concourse.bass_utils import run_bass_kernel_spmd

FP32 = mybir.dt.float32
FP32R = mybir.dt.float32r
BF16 = mybir.dt.bfloat16
AF = mybir.ActivationFunctionType

B, S, D, H, E = 4, 2048, 1024, 16, 64
NCORES = 8
TQ = S // 2  # query tokens per core
SCALE = 1.0 / float(np.sqrt(E))

_CACHE = {}


def build_nc():
    nc = bacc.Bacc("TRN2", target_bir_lowering=False)

    xT = nc.dram_tensor("xT", [D, S], FP32R, kind="ExternalInput")
    wq_t = nc.dram_tensor("wq_t", [D, H * E], FP32R, kind="ExternalInput")
    wk_t = nc.dram_tensor("wk_t", [D, H * E], FP32R, kind="ExternalInput")
    wv_t = nc.dram_tensor("wv_t", [D, H * E], FP32R, kind="ExternalInput")
    wo_t = nc.dram_tensor("wo_t", [D, D], FP32R, kind="ExternalInput")
    bqp = nc.dram_tensor("bqp", [128, 8], FP32, kind="ExternalInput")
    bkp = nc.dram_tensor("bkp", [128, 8], FP32, kind="ExternalInput")
    bv_row = nc.dram_tensor("bv_row", [1, H * E], FP32R, kind="ExternalInput")
    bo_row = nc.dram_tensor("bo_row", [1, D], FP32R, kind="ExternalInput")
    out = nc.dram_tensor("out", [TQ, D], FP32, kind="ExternalOutput")
    # V spill: [tok-tile, tok-in-tile, head, E+1]
    v_spill = nc.dram_tensor("v_spill", [16, 128, H, E + 1], BF16)

    xT_r = xT.rearrange("(t p) s -> p t s", p=128)  # [128, 8, 2048]
    wq_r = wq_t.rearrange("(t p) m -> p t m", p=128)  # [128, 8, 1024]
    wk_r = wk_t.rearrange("(t p) m -> p t m", p=128)
    wv_r = wv_t.rearrange("(t p) m -> p t m", p=128)
    wo_r = wo_t.rearrange("(t p) m -> p t m", p=128)

    with tile.TileContext(nc) as tc:
        with (
            tc.tile_pool(name="xt", bufs=1) as xt_pool,
            tc.tile_pool(name="wkq", bufs=2) as wkq_pool,
            tc.tile_pool(name="ktp", bufs=2) as kt_pool,
            tc.tile_pool(name="w256", bufs=3) as w256_pool,
            tc.tile_pool(name="qt", bufs=2) as qt_pool,
            tc.tile_pool(name="vbuf", bufs=2) as vbuf_pool,
            tc.tile_pool(name="vst", bufs=2) as vst_pool,
            tc.tile_pool(name="expp", bufs=3) as exp_pool,
            tc.tile_pool(name="attT", bufs=8) as attT_pool,
            tc.tile_pool(name="small", bufs=2) as small_pool,
            tc.tile_pool(name="ones", bufs=1) as ones_pool,
            tc.tile_pool(name="ps_s", bufs=2, space="PSUM") as ps_scores,
            tc.tile_pool(name="ps_a", bufs=2, space="PSUM") as ps_att,
            tc.tile_pool(name="ps_g", bufs=2, space="PSUM") as ps_gen,
        ):
            # ---- persistent tiles ----
            # 8 chunked loads so the first projection matmuls can start as
            # soon as their k-chunk lands instead of waiting for all 8MB
            xt_sb = xt_pool.tile([128, 8, S], FP32R, tag="xt")  # 64KB/part
            for k in range(8):
                nc.sync.dma_start(out=xt_sb[:, k, :], in_=xT_r[:, k, :])

            ones_col_f = ones_pool.tile([128, 8], FP32, tag="onescf")
            nc.vector.memset(ones_col_f, 1.0)
            ones_col = ones_pool.tile([128, 8], BF16, tag="onescol")
            nc.vector.tensor_copy(out=ones_col, in_=ones_col_f)
            bq_sb = ones_pool.tile([128, 8], FP32, tag="bq")
            bk_sb = ones_pool.tile([128, 8], FP32, tag="bk")
            nc.sync.dma_start(out=bq_sb, in_=bqp[:, :])
            nc.sync.dma_start(out=bk_sb, in_=bkp[:, :])

            # bias rows staged through the w256 pool (dead after broadcast)
            bv_sb = w256_pool.tile([1, H * E], FP32R, tag="w256", name="bvrow")
            bo_sb = w256_pool.tile([1, D], FP32R, tag="w256", name="borow")
            nc.sync.dma_start(out=bv_sb, in_=bv_row[:, :])
            nc.sync.dma_start(out=bo_sb, in_=bo_row[:, :])
            bv_bc = ones_pool.tile([128, H * E], FP32R, tag="bvbc")
            bo_bc = ones_pool.tile([128, D], FP32R, tag="bobc")
            nc.gpsimd.partition_broadcast(bv_bc, bv_sb)
            nc.gpsimd.partition_broadcast(bo_bc, bo_sb)

            attT_tiles = [
                attT_pool.tile([128, TQ], FP32R, tag="attT", name=f"attT{i}")
                for i in range(8)
            ]

            # ---- V projection for one head-quad (4 heads, 256 cols) ----
            def v_quad(vh):
                wv_sb = w256_pool.tile([128, 8, 256], FP32R, tag="w256")
                nc.sync.dma_start(out=wv_sb, in_=wv_r[:, :, vh * 256 : (vh + 1) * 256])
                for tokt in range(16):
                    ps = ps_gen.tile([128, 256], FP32, tag="gen")
                    for k in range(8):
                        nc.tensor.matmul(
                            out=ps,
                            lhsT=xt_sb[:, k, tokt * 128 : (tokt + 1) * 128],
                            rhs=wv_sb[:, k, :],
                            start=(k == 0),
                            stop=(k == 7),
                        )
                    vstage = vst_pool.tile([128, 4, E + 1], BF16, tag="vst")
                    nc.vector.tensor_add(
                        out=vstage[:, :, :E],
                        in0=ps.rearrange("p (h e) -> p h e", e=E),
                        in1=bv_bc[:, vh * 256 : (vh + 1) * 256].rearrange(
                            "p (h e) -> p h e", e=E
                        ),
                    )
                    nc.vector.tensor_copy(
                        out=vstage[:, :, E : E + 1], in_=ones_col[:, :4].unsqueeze(2)
                    )
                    nc.sync.dma_start(
                        out=v_spill[tokt, :, vh * 4 : (vh + 1) * 4, :], in_=vstage
                    )

            # heads 0-7 are needed from pass 0: project them up front
            v_quad(0)
            v_quad(1)

            # ---- passes: 2 heads each ----
            for p in range(8):
                wk_sb = wkq_pool.tile([128, 8, 128], FP32R, tag="wk")
                wq_sb = wkq_pool.tile([128, 8, 128], FP32R, tag="wq")
                nc.sync.dma_start(out=wk_sb, in_=wk_r[:, :, p * 128 : (p + 1) * 128])
                nc.sync.dma_start(out=wq_sb, in_=wq_r[:, :, p * 128 : (p + 1) * 128])

                kt_sb = kt_pool.tile([128, S], FP32R, tag="kt")
                qt_sb = qt_pool.tile([128, TQ], FP32R, tag="qt")

                for ts in range(4):
                    ps = ps_gen.tile([128, 512], FP32, tag="gen")
                    for k in range(8):
                        nc.tensor.matmul(
                            out=ps,
                            lhsT=wk_sb[:, k, :],
                            rhs=xt_sb[:, k, ts * 512 : (ts + 1) * 512],
                            start=(k == 0),
                            stop=(k == 7),
                        )
                    nc.vector.tensor_scalar_add(
                        out=kt_sb[:, ts * 512 : (ts + 1) * 512],
                        in0=ps,
                        scalar1=bk_sb[:, p : p + 1],
                    )
                for qs in range(2):
                    ps = ps_gen.tile([128, 512], FP32, tag="gen")
                    for k in range(8):
                        nc.tensor.matmul(
                            out=ps,
                            lhsT=wq_sb[:, k, :],
                            rhs=xt_sb[:, k, qs * 512 : (qs + 1) * 512],
                            start=(k == 0),
                            stop=(k == 7),
                        )
                    nc.vector.tensor_scalar_add(
                        out=qt_sb[:, qs * 512 : (qs + 1) * 512],
                        in0=ps,
                        scalar1=bq_sb[:, p : p + 1],
                    )

                for hh in range(2):
                    base = hh * 64
                    h = 2 * p + hh
                    vh_sb = vbuf_pool.tile([128, 16, E + 1], BF16, tag="vbuf")
                    nc.sync.dma_start(
                        out=vh_sb, in_=v_spill[:, :, h, :].transpose([1, 0, 2])
                    )
                    for tqt in range(2):
                        att_ps = ps_att.tile([E + 1, 512], FP32, tag="att")
                        for g in range(8):
                            ps_s = ps_scores.tile([128, 2, 512], FP32, tag="sc")
                            for j in range(2):
                                t = g * 2 + j
                                nc.tensor.matmul(
                                    out=ps_s[:, j, :],
                                    lhsT=kt_sb[
                                        base : base + 64, t * 128 : (t + 1) * 128
                                    ],
                                    rhs=qt_sb[
                                        base : base + 64, tqt * 512 : (tqt + 1) * 512
                                    ],
                                    start=True,
                                    stop=True,
                                )
                            exp_t = exp_pool.tile([128, 2, 512], BF16, tag="exp")
                            nc.scalar.activation(
                                out=exp_t, in_=ps_s, func=AF.Exp, scale=SCALE
                            )
                            for j in range(2):
                                t = g * 2 + j
                                nc.tensor.matmul(
                                    out=att_ps,
                                    lhsT=vh_sb[:, t, :],
                                    rhs=exp_t[:, j, :],
                                    start=(t == 0),
                                    stop=(t == 15),
                                )
                        # the custom-DVE reciprocal ignores the AP's partition
                        # offset, so first move the sums row (PSUM partition
                        # 64) to a partition-0 SBUF tile with a plain copy
                        sums_sb = small_pool.tile([1, 512], FP32, tag="sums", bufs=2)
                        nc.vector.tensor_copy(out=sums_sb, in_=att_ps[E : E + 1, :])
                        recip_r = small_pool.tile([1, 512], FP32, tag="recr", bufs=2)
                        recip_s = small_pool.tile([1, 512], FP32, tag="recs", bufs=2)
                        nc.vector.reciprocal_approx_accurate(
                            out=recip_r, in_=sums_sb, scratch=recip_s
                        )
                        rb_sb = small_pool.tile([64, 512], FP32, tag="rbb", bufs=2)
                        nc.gpsimd.partition_broadcast(rb_sb, recip_r)
                        nc.vector.tensor_mul(
                            out=attT_tiles[p][
                                base : base + 64, tqt * 512 : (tqt + 1) * 512
                            ],
                            in0=att_ps[:E, :],
                            in1=rb_sb,
                        )

                # defer V projection of heads 8-15 into the exp-bound passes:
                # quad 2 (heads 8-11) lands after pass 0, quad 3 after pass 1
                if p == 0:
                    v_quad(2)
                elif p == 1:
                    v_quad(3)

            # ---- output projection (4 col-quads of 256) ----
            for ohalf in range(4):
                wo_sb = w256_pool.tile([128, 8, 256], FP32R, tag="w256")
                nc.sync.dma_start(
                    out=wo_sb, in_=wo_r[:, :, ohalf * 256 : (ohalf + 1) * 256]
                )
                for tokt in range(8):
                    ps = ps_gen.tile([128, 256], FP32, tag="gen")
                    for t in range(8):
                        nc.tensor.matmul(
                            out=ps,
                            lhsT=attT_tiles[t][:, tokt * 128 : (tokt + 1) * 128],
                            rhs=wo_sb[:, t, :],
                            start=(t == 0),
                            stop=(t == 7),
                        )
                    ostg = small_pool.tile([128, 256], FP32, tag="stg", bufs=2)
                    nc.vector.tensor_add(
                        out=ostg, in0=ps, in1=bo_bc[:, ohalf * 256 : (ohalf + 1) * 256]
                    )
                    nc.sync.dma_start(
                        out=out[
                            tokt * 128 : (tokt + 1) * 128,
                            ohalf * 256 : (ohalf + 1) * 256,
                        ],
                        in_=ostg,
                    )

    nc.compile()
    return nc


def kernel(x, wq, bq, wk, bk, wv, bv, wo, bo, trace=False):
    x = np.asarray(x, dtype=np.float32)
    wq = np.asarray(wq, dtype=np.float32)
    bq = np.asarray(bq, dtype=np.float32)
    wk = np.asarray(wk, dtype=np.float32)
    bk = np.asarray(bk, dtype=np.float32)
    wv = np.asarray(wv, dtype=np.float32)
    bv = np.asarray(bv, dtype=np.float32)
    wo = np.asarray(wo, dtype=np.float32)
    bo = np.asarray(bo, dtype=np.float32)

    if "nc" not in _CACHE:
        _CACHE["nc"] = build_nc()
    nc = _CACHE["nc"]

    wq_t = np.ascontiguousarray(wq.transpose(1, 0, 2).reshape(D, H * E))
    wk_t = np.ascontiguousarray(wk.transpose(1, 0, 2).reshape(D, H * E))
    wv_t = np.ascontiguousarray(wv.transpose(1, 0, 2).reshape(D, H * E))
    wo_t = np.ascontiguousarray(wo.T)
    bqp = np.ascontiguousarray(bq.reshape(H * E).reshape(8, 128).T)
    bkp = np.ascontiguousarray(bk.reshape(H * E).reshape(8, 128).T)
    bv_row = np.ascontiguousarray(bv.reshape(1, H * E))
    bo_row = np.ascontiguousarray(bo.reshape(1, D))

    shared = {
        "wq_t": wq_t,
        "wk_t": wk_t,
        "wv_t": wv_t,
        "wo_t": wo_t,
        "bqp": bqp,
        "bkp": bkp,
        "bv_row": bv_row,
        "bo_row": bo_row,
    }
    in_maps = []
    for c in range(NCORES):
        b, qh = c // 2, c % 2
        xT_b = x[b].T  # [D, S]
        if qh == 0:
            xT_c = np.ascontiguousarray(xT_b)
        else:
            # my query tokens first; K/V token order is irrelevant as long as
            # K and V agree (softmax + weighted sum are permutation-invariant)
            xT_c = np.ascontiguousarray(
                np.concatenate([xT_b[:, TQ:], xT_b[:, :TQ]], axis=1)
            )
        m = dict(shared)
        m["xT"] = xT_c
        in_maps.append(m)

    res = run_bass_kernel_spmd(nc, in_maps, list(range(NCORES)), trace=trace)

    out = np.empty((B, S, D), dtype=np.float32)
    for c in range(NCORES):
        b, qh = c // 2, c % 2
        out[b, qh * TQ : (qh + 1) * TQ, :] = res.results[c]["out"]
    if trace:
        return out, res
    return out


# revision 19
# speedup vs baseline: 1.3170x; 1.0919x over previous
"""Multi-head attention (B=4, S=2048, D=1024, H=16, E=64) on 8 TRN2 NeuronCores.

Sharding: core c handles batch b=c//2 and query-half qh=c%2 (1024 query tokens).
K/V are computed per-core for the full 2048-token sequence of its batch (2x
duplicated K/V projection work, but zero collectives / zero cross-core deps).

The host reorders each core's xT so the core's OWN query tokens occupy columns
0..1023 (attention is invariant to key/value token order as long as K and V
share it), so a single SPMD program serves all cores without a separate xTq
input.

Per-core program (SPMD, identical on all cores):
  V projection (4 head-quads of 256 cols): V = x @ wv + bv, stored
    [tok, head, 65] with a ones-column per head (softmax sums fall out of the
    att@V matmul), spilled to DRAM scratch. Quads 0-1 run up front; quads 2-3
    (heads 8-15, first needed in pass 4) are emitted after passes 0/1 so the
    Tile scheduler uses them as PE filler inside the exp-bound inner loop.
  passes p=0..7 (heads 2p, 2p+1):
    KT[128he, 2048tok] = (wk_p.T @ xT) + bk  (transposed layout, bias on DVE)
    QT[128he, 1024tq]  = (wq_p.T @ xT[:, :1024]) + bq
    per (head, tq-tile of 512):
      scoresT[tk,tq] = KT_h.T-slices @ QT_h  (K=64 matmuls, one per tk-tile)
      exp on ScalarE straight from PSUM with scale=1/8 (softmax max-subtraction
      skipped: |score/8| <= ~12 so exp is fp32-safe)
      attT[65,tq] += [V_h | 1].T @ expT  (row 64 accumulates the softmax sum)
      normalize: reciprocal_approx on VectorE, partition_broadcast on GpSimd,
      multiply on VectorE — the PE is never involved
  output projection (4 col-quads of 256): out = att @ wo.T + bo, bias added on
    DVE from a partition-broadcast bo tile.

Double-buffered wkq/kt/qt pools let the scheduler run the NEXT pass's K/Q
projections as PE filler during the current pass's attention inner loop, which
is otherwise rate-limited by ScalarE exp (~1.1us per 2-ktile group) and stalls
the PE, dropping its p-state.

All matmuls run in float32r (tf32-like, full PE rate at N>=256).
"""

import numpy as np

import concourse.bass as bass
import concourse.mybir as mybir
import concourse.tile as tile
from concourse import bacc
from concourse.bass_utils import run_bass_kernel_spmd

FP32 = mybir.dt.float32
FP32R = mybir.dt.float32r
BF16 = mybir.dt.bfloat16
AF = mybir.ActivationFunctionType

B, S, D, H, E = 4, 2048, 1024, 16, 64
NCORES = 8
TQ = S // 2  # query tokens per core
SCALE = 1.0 / float(np.sqrt(E))

_CACHE = {}


def build_nc():
    nc = bacc.Bacc("TRN2", target_bir_lowering=False)

    xT = nc.dram_tensor("xT", [D, S], FP32R, kind="ExternalInput")
    wq_t = nc.dram_tensor("wq_t", [D, H * E], FP32R, kind="ExternalInput")
    wk_t = nc.dram_tensor("wk_t", [D, H * E], FP32R, kind="ExternalInput")
    wv_t = nc.dram_tensor("wv_t", [D, H * E], FP32R, kind="ExternalInput")
    wo_t = nc.dram_tensor("wo_t", [D, D], FP32R, kind="ExternalInput")
    bqp = nc.dram_tensor("bqp", [128, 8], FP32, kind="ExternalInput")
    bkp = nc.dram_tensor("bkp", [128, 8], FP32, kind="ExternalInput")
    bv_row = nc.dram_tensor("bv_row", [1, H * E], FP32R, kind="ExternalInput")
    bo_row = nc.dram_tensor("bo_row", [1, D], FP32R, kind="ExternalInput")
    out = nc.dram_tensor("out", [TQ, D], FP32, kind="ExternalOutput")
    # V spill: [tok-tile, tok-in-tile, head, E+1]
    v_spill = nc.dram_tensor("v_spill", [16, 128, H, E + 1], BF16)

    xT_r = xT.rearrange("(t p) s -> p t s", p=128)  # [128, 8, 2048]
    wq_r = wq_t.rearrange("(t p) m -> p t m", p=128)  # [128, 8, 1024]
    wk_r = wk_t.rearrange("(t p) m -> p t m", p=128)
    wv_r = wv_t.rearrange("(t p) m -> p t m", p=128)
    wo_r = wo_t.rearrange("(t p) m -> p t m", p=128)

    with tile.TileContext(nc) as tc:
        with (
            tc.tile_pool(name="xt", bufs=1) as xt_pool,
            tc.tile_pool(name="wkq", bufs=2) as wkq_pool,
            tc.tile_pool(name="ktp", bufs=2) as kt_pool,
            tc.tile_pool(name="w256", bufs=4) as w256_pool,
            tc.tile_pool(name="qt", bufs=2) as qt_pool,
            tc.tile_pool(name="vbuf", bufs=2) as vbuf_pool,
            tc.tile_pool(name="vst", bufs=2) as vst_pool,
            tc.tile_pool(name="expp", bufs=4) as exp_pool,
            tc.tile_pool(name="attT", bufs=8) as attT_pool,
            tc.tile_pool(name="small", bufs=2) as small_pool,
            tc.tile_pool(name="ones", bufs=1) as ones_pool,
            tc.tile_pool(name="ps_s", bufs=2, space="PSUM") as ps_scores,
            tc.tile_pool(name="ps_a", bufs=2, space="PSUM") as ps_att,
            tc.tile_pool(name="ps_g", bufs=2, space="PSUM") as ps_gen,
        ):
            # ---- persistent tiles ----
            # 8 chunked loads so the first projection matmuls can start as
            # soon as their k-chunk lands instead of waiting for all 8MB
            xt_sb = xt_pool.tile([128, 8, S], FP32R, tag="xt")  # 64KB/part
            for k in range(8):
                nc.sync.dma_start(out=xt_sb[:, k, :], in_=xT_r[:, k, :])

            ones_col_f = ones_pool.tile([128, 8], FP32, tag="onescf")
            nc.vector.memset(ones_col_f, 1.0)
            ones_col = ones_pool.tile([128, 8], BF16, tag="onescol")
            nc.vector.tensor_copy(out=ones_col, in_=ones_col_f)
            bq_sb = ones_pool.tile([128, 8], FP32, tag="bq")
            bk_sb = ones_pool.tile([128, 8], FP32, tag="bk")
            nc.sync.dma_start(out=bq_sb, in_=bqp[:, :])
            nc.sync.dma_start(out=bk_sb, in_=bkp[:, :])

            # bias rows staged through the w256 pool (dead after broadcast)
            bv_sb = w256_pool.tile([1, H * E], FP32R, tag="w256", name="bvrow")
            bo_sb = w256_pool.tile([1, D], FP32R, tag="w256", name="borow")
            nc.sync.dma_start(out=bv_sb, in_=bv_row[:, :])
            nc.sync.dma_start(out=bo_sb, in_=bo_row[:, :])
            bv_bc = ones_pool.tile([128, H * E], FP32R, tag="bvbc")
            bo_bc = ones_pool.tile([128, D], FP32R, tag="bobc")
            nc.gpsimd.partition_broadcast(bv_bc, bv_sb)
            nc.gpsimd.partition_broadcast(bo_bc, bo_sb)

            attT_tiles = [
                attT_pool.tile([128, TQ], FP32R, tag="attT", name=f"attT{i}")
                for i in range(8)
            ]

            # ---- V projection for one head-quad (4 heads, 256 cols) ----
            def v_quad(vh):
                wv_sb = w256_pool.tile([128, 8, 256], FP32R, tag="w256")
                nc.sync.dma_start(out=wv_sb, in_=wv_r[:, :, vh * 256 : (vh + 1) * 256])
                for tokt in range(16):
                    ps = ps_gen.tile([128, 256], FP32, tag="gen")
                    for k in range(8):
                        nc.tensor.matmul(
                            out=ps,
                            lhsT=xt_sb[:, k, tokt * 128 : (tokt + 1) * 128],
                            rhs=wv_sb[:, k, :],
                            start=(k == 0),
                            stop=(k == 7),
                        )
                    vstage = vst_pool.tile([128, 4, E + 1], BF16, tag="vst")
                    nc.vector.tensor_add(
                        out=vstage[:, :, :E],
                        in0=ps.rearrange("p (h e) -> p h e", e=E),
                        in1=bv_bc[:, vh * 256 : (vh + 1) * 256].rearrange(
                            "p (h e) -> p h e", e=E
                        ),
                    )
                    nc.vector.tensor_copy(
                        out=vstage[:, :, E : E + 1], in_=ones_col[:, :4].unsqueeze(2)
                    )
                    nc.sync.dma_start(
                        out=v_spill[tokt, :, vh * 4 : (vh + 1) * 4, :], in_=vstage
                    )

            # heads 0-7 are needed from pass 0: project them up front
            v_quad(0)
            v_quad(1)

            # ---- passes: 2 heads each ----
            for p in range(8):
                wk_sb = wkq_pool.tile([128, 8, 128], FP32R, tag="wk")
                wq_sb = wkq_pool.tile([128, 8, 128], FP32R, tag="wq")
                nc.sync.dma_start(out=wk_sb, in_=wk_r[:, :, p * 128 : (p + 1) * 128])
                nc.sync.dma_start(out=wq_sb, in_=wq_r[:, :, p * 128 : (p + 1) * 128])

                kt_sb = kt_pool.tile([128, S], FP32R, tag="kt")
                qt_sb = qt_pool.tile([128, TQ], FP32R, tag="qt")

                for ts in range(4):
                    ps = ps_gen.tile([128, 512], FP32, tag="gen")
                    for k in range(8):
                        nc.tensor.matmul(
                            out=ps,
                            lhsT=wk_sb[:, k, :],
                            rhs=xt_sb[:, k, ts * 512 : (ts + 1) * 512],
                            start=(k == 0),
                            stop=(k == 7),
                        )
                    nc.vector.tensor_scalar_add(
                        out=kt_sb[:, ts * 512 : (ts + 1) * 512],
                        in0=ps,
                        scalar1=bk_sb[:, p : p + 1],
                    )
                for qs in range(2):
                    ps = ps_gen.tile([128, 512], FP32, tag="gen")
                    for k in range(8):
                        nc.tensor.matmul(
                            out=ps,
                            lhsT=wq_sb[:, k, :],
                            rhs=xt_sb[:, k, qs * 512 : (qs + 1) * 512],
                            start=(k == 0),
                            stop=(k == 7),
                        )
                    nc.vector.tensor_scalar_add(
                        out=qt_sb[:, qs * 512 : (qs + 1) * 512],
                        in0=ps,
                        scalar1=bq_sb[:, p : p + 1],
                    )

                for hh in range(2):
                    base = hh * 64
                    h = 2 * p + hh
                    vh_sb = vbuf_pool.tile([128, 16, E + 1], BF16, tag="vbuf")
                    nc.sync.dma_start(
                        out=vh_sb, in_=v_spill[:, :, h, :].transpose([1, 0, 2])
                    )
                    for tqt in range(2):
                        att_ps = ps_att.tile([E + 1, 512], FP32, tag="att")

                        def att_group(gg, exp_t):
                            for j in range(2):
                                t = gg * 2 + j
                                nc.tensor.matmul(
                                    out=att_ps,
                                    lhsT=vh_sb[:, t, :],
                                    rhs=exp_t[:, j, :],
                                    start=(t == 0),
                                    stop=(t == 15),
                                )

                        # att@V lags the exp by 2 groups so the baked static
                        # PE order has ~2us of slack on each exp dependency
                        # (runtime exp jitter otherwise stalls the PE and
                        # drops its p-state)
                        exp_tiles = []
                        for g in range(8):
                            ps_s = ps_scores.tile([128, 2, 512], FP32, tag="sc")
                            for j in range(2):
                                t = g * 2 + j
                                nc.tensor.matmul(
                                    out=ps_s[:, j, :],
                                    lhsT=kt_sb[
                                        base : base + 64, t * 128 : (t + 1) * 128
                                    ],
                                    rhs=qt_sb[
                                        base : base + 64, tqt * 512 : (tqt + 1) * 512
                                    ],
                                    start=True,
                                    stop=True,
                                )
                            exp_t = exp_pool.tile([128, 2, 512], BF16, tag="exp")
                            nc.scalar.activation(
                                out=exp_t, in_=ps_s, func=AF.Exp, scale=SCALE
                            )
                            exp_tiles.append(exp_t)
                            if g >= 2:
                                att_group(g - 2, exp_tiles[g - 2])
                        att_group(6, exp_tiles[6])
                        att_group(7, exp_tiles[7])
                        # the custom-DVE reciprocal ignores the AP's partition
                        # offset, so first move the sums row (PSUM partition
                        # 64) to a partition-0 SBUF tile with a plain copy
                        sums_sb = small_pool.tile([1, 512], FP32, tag="sums", bufs=2)
                        nc.vector.tensor_copy(out=sums_sb, in_=att_ps[E : E + 1, :])
                        recip_r = small_pool.tile([1, 512], FP32, tag="recr", bufs=2)
                        recip_s = small_pool.tile([1, 512], FP32, tag="recs", bufs=2)
                        nc.vector.reciprocal_approx_accurate(
                            out=recip_r, in_=sums_sb, scratch=recip_s
                        )
                        rb_sb = small_pool.tile([64, 512], FP32, tag="rbb", bufs=2)
                        nc.gpsimd.partition_broadcast(rb_sb, recip_r)
                        nc.vector.tensor_mul(
                            out=attT_tiles[p][
                                base : base + 64, tqt * 512 : (tqt + 1) * 512
                            ],
                            in0=att_ps[:E, :],
                            in1=rb_sb,
                        )

                # defer V projection of heads 8-15 into the exp-bound passes:
                # quad 2 (heads 8-11) lands after pass 0, quad 3 after pass 1
                if p == 0:
                    v_quad(2)
                elif p == 1:
                    v_quad(3)

            # ---- output projection (4 col-quads of 256) ----
            wo_sbs = []
            for ohalf in range(4):
                wo_sb = w256_pool.tile(
                    [128, 8, 256], FP32R, tag="w256", name=f"wo{ohalf}"
                )
                nc.sync.dma_start(
                    out=wo_sb, in_=wo_r[:, :, ohalf * 256 : (ohalf + 1) * 256]
                )
                wo_sbs.append(wo_sb)
            for ohalf in range(4):
                wo_sb = wo_sbs[ohalf]
                for tokt in range(8):
                    ps = ps_gen.tile([128, 256], FP32, tag="gen")
                    for t in range(8):
                        nc.tensor.matmul(
                            out=ps,
                            lhsT=attT_tiles[t][:, tokt * 128 : (tokt + 1) * 128],
                            rhs=wo_sb[:, t, :],
                            start=(t == 0),
                            stop=(t == 7),
                        )
                    ostg = small_pool.tile([128, 256], FP32, tag="stg", bufs=2)
                    nc.vector.tensor_add(
                        out=ostg, in0=ps, in1=bo_bc[:, ohalf * 256 : (ohalf + 1) * 256]
                    )
                    nc.sync.dma_start(
                        out=out[
                            tokt * 128 : (tokt + 1) * 128,
                            ohalf * 256 : (ohalf + 1) * 256,
                        ],
                        in_=ostg,
                    )

    nc.compile()
    return nc


def kernel(x, wq, bq, wk, bk, wv, bv, wo, bo, trace=False):
    x = np.asarray(x, dtype=np.float32)
    wq = np.asarray(wq, dtype=np.float32)
    bq = np.asarray(bq, dtype=np.float32)
    wk = np.asarray(wk, dtype=np.float32)
    bk = np.asarray(bk, dtype=np.float32)
    wv = np.asarray(wv, dtype=np.float32)
    bv = np.asarray(bv, dtype=np.float32)
    wo = np.asarray(wo, dtype=np.float32)
    bo = np.asarray(bo, dtype=np.float32)

    if "nc" not in _CACHE:
        _CACHE["nc"] = build_nc()
    nc = _CACHE["nc"]

    wq_t = np.ascontiguousarray(wq.transpose(1, 0, 2).reshape(D, H * E))
    wk_t = np.ascontiguousarray(wk.transpose(1, 0, 2).reshape(D, H * E))
    wv_t = np.ascontiguousarray(wv.transpose(1, 0, 2).reshape(D, H * E))
    wo_t = np.ascontiguousarray(wo.T)
    bqp = np.ascontiguousarray(bq.reshape(H * E).reshape(8, 128).T)
    bkp = np.ascontiguousarray(bk.reshape(H * E).reshape(8, 128).T)
    bv_row = np.ascontiguousarray(bv.reshape(1, H * E))
    bo_row = np.ascontiguousarray(bo.reshape(1, D))

    shared = {
        "wq_t": wq_t,
        "wk_t": wk_t,
        "wv_t": wv_t,
        "wo_t": wo_t,
        "bqp": bqp,
        "bkp": bkp,
        "bv_row": bv_row,
        "bo_row": bo_row,
    }
    in_maps = []
    for c in range(NCORES):
        b, qh = c // 2, c % 2
        xT_b = x[b].T  # [D, S]
        if qh == 0:
            xT_c = np.ascontiguousarray(xT_b)
        else:
            # my query tokens first; K/V token order is irrelevant as long as
            # K and V agree (softmax + weighted sum are permutation-invariant)
            xT_c = np.ascontiguousarray(
                np.concatenate([xT_b[:, TQ:], xT_b[:, :TQ]], axis=1)
            )
        m = dict(shared)
        m["xT"] = xT_c
        in_maps.append(m)

    res = run_bass_kernel_spmd(nc, in_maps, list(range(NCORES)), trace=trace)

    out = np.empty((B, S, D), dtype=np.float32)
    for c in range(NCORES):
        b, qh = c // 2, c % 2
        out[b, qh * TQ : (qh + 1) * TQ, :] = res.results[c]["out"]
    if trace:
        return out, res
    return out
